# revision 3
# baseline (speedup 1.0000x reference)
"""DMN encoder (3-hop masked-attention message passing) on 8 trn2 cores.

Sharding: pure data-parallel over the batch dim (16 rows/core). Each core:
  - loads its value slice as bf16 (SWDGE cast during DMA),
  - computes hop-invariant per-neighbor dots vs = V.wf, vu = V.wu via
    PE-transpose + matmul,
  - runs the 3-hop recurrence where softmax numerators are
    num_h = mask * max(exp(vs) * exp(c_h), 1)   (exact relu-softmax identity),
    and hop-1's scalar c_1 needs only sum(num0*vu)/D0 (no [B,N,D] pass),
  - streams V twice through the PE: once for (o0, o1) together, once for o2.
"""
import sys

sys.path.insert(0, "/opt/trn_rl_repo")

import numpy as np
import concourse.bass as bass
import concourse.tile as tile
from concourse import mybir
from concourse.bass_utils import run_bass_kernel_spmd
from contextlib import ExitStack

N_CORES = 8
B, N, D = 128, 2048, 128
BC = B // N_CORES          # batch rows per core
CH = N // 128              # neighbor chunks of 128
AF = mybir.ActivationFunctionType
ALU = mybir.AluOpType
FP32 = mybir.dt.float32
BF16 = mybir.dt.bfloat16
BIG = 3.0e4                # mask-out offset for the m1 max (fp32-safe)
CLAMP = 60.0               # overflow guard on exp() arguments

_mwctr = [0]


def _split_multiwaits(nc):
    """This walrus build rejects >1 sync-wait per instruction; hoist extras
    onto standalone EventSemaphore instructions on the same engine."""
    for fn in nc.m.functions:
        for bb in fn.blocks:
            new_list = []
            changed = False
            for ins in bb.instructions:
                si = getattr(ins, "sync_info", None)
                on_wait = list(si.on_wait) if si is not None else []
                if len(on_wait) > 1:
                    changed = True
                    for w in on_wait[:-1]:
                        _mwctr[0] += 1
                        ev = mybir.InstEventSemaphore(
                            name=f"I-mwfix-{_mwctr[0]}", ins=[], outs=[])
                        ev.engine = ins.engine
                        ev.debug = ins.debug
                        ev.sync_info = mybir.SyncInfo(on_wait=[w], on_update=[])
                        new_list.append(ev)
                        nc.register_instruction(ev, overwrite=True)
                    si.on_wait = [on_wait[-1]]
                    ins.sync_info = si
                new_list.append(ins)
            if changed:
                live = bb.instructions
                live[:] = new_list


def _build():
    nc = bass.Bass()
    value = nc.dram_tensor("value", [BC, N, D], FP32, kind="ExternalInput")
    mask_t = nc.dram_tensor("mask_t", [N, BC], FP32, kind="ExternalInput")
    e1_t = nc.dram_tensor("e1_t", [D, BC], FP32, kind="ExternalInput")
    w_lhsT = nc.dram_tensor("w_lhsT", [D, D], FP32, kind="ExternalInput")
    b_col = nc.dram_tensor("b_col", [D, 1], FP32, kind="ExternalInput")
    wfu_in = nc.dram_tensor("wfu", [D, 2], FP32, kind="ExternalInput")
    attb_in = nc.dram_tensor("attb", [1, 1], FP32, kind="ExternalInput")
    ident_in = nc.dram_tensor("ident", [128, 128], FP32, kind="ExternalInput")
    y = nc.dram_tensor("y", [BC, D], FP32, kind="ExternalOutput")

    with tile.TileContext(nc) as tc, ExitStack() as ctx:
        P = lambda **kw: ctx.enter_context(tc.tile_pool(**kw))
        sb = P(name="sb", bufs=1)                       # persistent singles
        vt = P(name="vt", bufs=3)                       # transpose staging
        wk = P(name="wk", bufs=2)                       # temporaries
        ps_tr = P(name="ps_tr", bufs=2, space="PSUM")   # transpose batches
        ps_acc = P(name="ps_acc", bufs=2, space="PSUM")  # accumulators (vs/vu + pass A/B)
        ps_sm = P(name="ps_sm", bufs=2, space="PSUM")   # small matmul outs

        # ---- init: small params ----
        w_sb = sb.tile([D, D], FP32, tag="w_sb")
        nc.sync.dma_start(out=w_sb, in_=w_lhsT[:, :])
        bcol_sb = sb.tile([D, 1], FP32, tag="bcol")
        nc.sync.dma_start(out=bcol_sb, in_=b_col[:, :])
        wfu_sb = sb.tile([D, 2], FP32, tag="wfu")
        nc.sync.dma_start(out=wfu_sb, in_=wfu_in[:, :])
        attb_sb = sb.tile([1, 1], FP32, tag="attb")
        nc.sync.dma_start(out=attb_sb, in_=attb_in[:, :])
        identf = sb.tile([128, 128], FP32, tag="identf")
        nc.sync.dma_start(out=identf, in_=ident_in[:, :])
        u0 = sb.tile([D, BC], FP32, tag="u0")
        nc.sync.dma_start(out=u0, in_=e1_t[:, :])

        identb = sb.tile([128, 128], BF16, tag="identb")
        nc.vector.tensor_copy(identb, identf)
        wfu_bf = sb.tile([D, 2], BF16, tag="wfub")
        nc.vector.tensor_copy(wfu_bf, wfu_sb)
        ones_col = sb.tile([128, 1], FP32, tag="onesc")
        nc.vector.memset(ones_col, 1.0)
        ones_row = sb.tile([1, 128], FP32, tag="onesr")
        nc.vector.memset(ones_row, 1.0)

        # mask: [N, BC] fp32 -> [128n, CH, BC] bf16
        mask_sb = sb.tile([128, CH, BC], BF16, tag="mask")
        nc.gpsimd.dma_start(
            out=mask_sb,
            in_=mask_t.rearrange("(c p) b -> p c b", p=128))

        # ---- V loads (bf16 cast) ----
        v_sb = []
        for b in range(BC):
            vtile = sb.tile([128, CH, D], BF16, tag=f"v{b}")
            nc.gpsimd.dma_start(
                out=vtile,
                in_=value[b].rearrange("(c p) d -> p c d", p=128))
            v_sb.append(vtile)

        # ---- vs/vu: per chunk PE-transpose then x[wf|wu] ----
        vsvu = sb.tile([128, CH, BC, 2], FP32, tag="vsvu")
        for b in range(BC):
            acc_vv = ps_acc.tile([128, 2 * CH], FP32, tag="acc")
            for cg in range(CH // 4):
                tr = ps_tr.tile([128, 512], BF16, tag="tr")
                for i in range(4):
                    c = cg * 4 + i
                    nc.tensor.transpose(
                        out=tr[:, i * 128:(i + 1) * 128],
                        in_=v_sb[b][:, c, :],
                        identity=identb)
                vt4 = vt.tile([128, 512], BF16, tag="vt4")
                if cg % 2 == 0:
                    nc.vector.tensor_copy(vt4, tr)
                else:
                    nc.scalar.activation(out=vt4, in_=tr, func=AF.Copy)
                for i in range(4):
                    c = cg * 4 + i
                    nc.tensor.matmul(
                        acc_vv[:, c * 2:(c + 1) * 2],
                        lhsT=vt4[:, i * 128:(i + 1) * 128],
                        rhs=wfu_bf,
                        start=True, stop=True)
            # scatter [128, (c,2)] -> vsvu[:, :, b, :]
            nc.vector.tensor_copy(
                vsvu[:, :, b, :],
                acc_vv.rearrange("p (c h) -> p c h", h=2))

        vs_view = vsvu[:, :, :, 0]
        vu_view = vsvu[:, :, :, 1]

        # ---- E = exp(vs), m1 = max over masked n of vs ----
        E = sb.tile([128, CH, BC], BF16, tag="E")
        nc.scalar.activation(out=E, in_=vs_view, func=AF.Exp)

        s1 = wk.tile([128, CH, BC], FP32, tag="s1")
        nc.vector.tensor_tensor(out=s1, in0=vs_view, in1=mask_sb, op=ALU.mult)
        nc.vector.scalar_tensor_tensor(
            out=s1, in0=mask_sb, scalar=BIG, in1=s1, op0=ALU.mult, op1=ALU.add)
        nc.vector.tensor_scalar_add(s1, s1, -BIG)
        red_m = wk.tile([128, BC], FP32, tag="redm")
        nc.vector.tensor_reduce(
            out=red_m, in_=s1.rearrange("p c b -> p b c"),
            axis=mybir.AxisListType.X, op=ALU.max)
        m1 = sb.tile([1, BC], FP32, tag="m1")
        nc.gpsimd.tensor_reduce(
            out=m1, in_=red_m, axis=mybir.AxisListType.C, op=ALU.max)

        # ---- helpers for the scalar chain ----
        def dot_wu(rhs_tile, name):
            """[1, BC] psum = wu . rhs  (+attb added by caller)"""
            ps = ps_sm.tile([1, BC], FP32, tag="sm1")
            nc.tensor.matmul(ps, lhsT=wfu_sb[:, 1:2], rhs=rhs_tile,
                             start=True, stop=True)
            return ps

        def bcast_row(src_1xbc, tag):
            """[128, BC] psum = broadcast of [1, BC] across partitions."""
            ps = ps_sm.tile([128, BC], FP32, tag="smb")
            nc.tensor.matmul(ps, lhsT=ones_row, rhs=src_1xbc,
                             start=True, stop=True)
            return ps

        def colsum(red_tile):
            """[1, BC] psum = column sums of [128, BC]."""
            ps = ps_sm.tile([1, BC], FP32, tag="sm1")
            nc.tensor.matmul(ps, lhsT=ones_col, rhs=red_tile,
                             start=True, stop=True)
            return ps

        def hop_scalars(c_sb, h):
            """Given c [1,BC] in SBUF: returns (t_bc psum [128,BC],
            corr [1,BC] sbuf)."""
            a = wk.tile([1, BC], FP32, tag=f"hs_a{h}")
            nc.vector.tensor_tensor(out=a, in0=m1, in1=c_sb, op=ALU.add)
            nc.vector.tensor_scalar_max(a, a, 0.0)
            r2 = wk.tile([1, BC], FP32, tag=f"hs_r{h}")
            nc.vector.tensor_scalar_max(r2, c_sb, 0.0)
            nc.vector.tensor_tensor(out=a, in0=a, in1=r2, op=ALU.max)
            nc.vector.tensor_scalar_min(a, a, CLAMP)
            corr = sb.tile([1, BC], FP32, tag=f"corr{h}")
            nc.scalar.activation(out=corr, in_=a, func=AF.Exp)
            nc.vector.tensor_scalar_mul(corr, corr, 1.0e-5)
            tcl = wk.tile([1, BC], FP32, tag=f"hs_t{h}")
            nc.vector.tensor_scalar_min(tcl, c_sb, CLAMP)
            texp = sb.tile([1, BC], FP32, tag=f"texp{h}")
            nc.scalar.activation(out=texp, in_=tcl, func=AF.Exp)
            return bcast_row(texp, f"tbc{h}"), corr

        def make_num(t_bc, num_out_view, h):
            """num = mask * max(E * t, 1) -> num_out_view (bf16)."""
            tmp = wk.tile([128, CH, BC], BF16, tag=f"numt{h}")
            nc.vector.tensor_tensor(
                out=tmp, in0=E,
                in1=bass.AP(tensor=t_bc.tensor, offset=t_bc.offset,
                            ap=[t_bc.ap[0], [0, CH], t_bc.ap[1]]),
                op=ALU.mult)
            nc.vector.tensor_scalar_max(tmp, tmp, 1.0)
            nc.vector.tensor_tensor(
                out=num_out_view, in0=tmp, in1=mask_sb, op=ALU.mult)

        def denom(num_view, corr, h):
            """D = colsum(num) + corr; returns (D_sb, recip_sb) [1,BC]."""
            red = wk.tile([128, BC], FP32, tag=f"dred{h}")
            nc.vector.tensor_reduce(
                out=red, in_=num_view.rearrange("p c b -> p b c"),
                axis=mybir.AxisListType.X, op=ALU.add)
            ps = colsum(red)
            d_sb = sb.tile([1, BC], FP32, tag=f"D{h}")
            nc.vector.tensor_tensor(out=d_sb, in0=ps, in1=corr, op=ALU.add)
            recip = sb.tile([1, BC], FP32, tag=f"recip{h}")
            nc.vector.reciprocal(recip, d_sb)
            return d_sb, recip

        def weighted_sum_vu(num_view, recip, h):
            """[1, BC] sbuf = sum(num * vu) * recip."""
            nv = wk.tile([128, CH, BC], FP32, tag=f"nv{h}")
            nc.vector.tensor_tensor(out=nv, in0=num_view, in1=vu_view,
                                    op=ALU.mult)
            red = wk.tile([128, BC], FP32, tag=f"nvred{h}")
            nc.vector.tensor_reduce(
                out=red, in_=nv.rearrange("p c b -> p b c"),
                axis=mybir.AxisListType.X, op=ALU.add)
            ps = colsum(red)
            out = sb.tile([1, BC], FP32, tag=f"owu{h}")
            nc.vector.tensor_tensor(out=out, in0=ps, in1=recip, op=ALU.mult)
            return out

        def lin_relu(u_tile, h):
            """ub = relu(W @ u + b) [D, BC]."""
            ps = ps_sm.tile([D, BC], FP32, tag="smb")
            nc.tensor.matmul(ps, lhsT=w_sb, rhs=u_tile, start=True, stop=True)
            ub = sb.tile([D, BC], FP32, tag=f"ub{h}")
            nc.scalar.activation(out=ub, in_=ps, func=AF.Relu,
                                 bias=bcol_sb, scale=1.0)
            return ub

        def make_c(base_u, owu, h):
            """c = wu . base_u + attb (+ owu) -> [1, BC] sbuf."""
            ps = dot_wu(base_u, h)
            c_sb = sb.tile([1, BC], FP32, tag=f"c{h}")
            nc.vector.tensor_scalar(
                out=c_sb, in0=ps, scalar1=attb_sb, scalar2=None,
                op0=ALU.add)
            if owu is not None:
                nc.vector.tensor_tensor(out=c_sb, in0=c_sb, in1=owu,
                                        op=ALU.add)
            return c_sb

        # ---- hops 0 and 1 (numerators only need scalars) ----
        num01 = sb.tile([128, CH, BC, 2], BF16, tag="num01")

        c0 = make_c(u0, None, 0)
        t0bc, corr0 = hop_scalars(c0, 0)
        make_num(t0bc, num01[:, :, :, 0], 0)
        d0, recip0 = denom(num01[:, :, :, 0], corr0, 0)
        o0wu = weighted_sum_vu(num01[:, :, :, 0], recip0, 0)

        ub0 = lin_relu(u0, 0)
        c1 = make_c(ub0, o0wu, 1)
        t1bc, corr1 = hop_scalars(c1, 1)
        make_num(t1bc, num01[:, :, :, 1], 1)
        d1, recip1 = denom(num01[:, :, :, 1], corr1, 1)
        o1wu = weighted_sum_vu(num01[:, :, :, 1], recip1, 1)

        # ---- pass A: o0, o1 together ----
        o01 = sb.tile([128, BC, 2], FP32, tag="o01")
        for b in range(BC):
            acc = ps_acc.tile([2, 128], FP32, tag="acc")
            for c in range(CH):
                nc.tensor.matmul(
                    acc, lhsT=num01[:, c, b, :], rhs=v_sb[b][:, c, :],
                    start=(c == 0), stop=(c == CH - 1))
            oa_sb = wk.tile([2, 128], FP32, tag="oa")
            if b % 2 == 0:
                nc.vector.tensor_copy(oa_sb, acc)
            else:
                nc.scalar.activation(out=oa_sb, in_=acc, func=AF.Copy)
            ps_t = ps_sm.tile([128, 2], FP32, tag="smb")
            nc.tensor.transpose(out=ps_t, in_=oa_sb,
                                identity=identf[0:2, 0:2])
            nc.vector.tensor_copy(o01[:, b, :], ps_t)

        # ---- u1, u2, hop-2 numerators ----
        r0bc = bcast_row(recip0, "r0")
        u1 = sb.tile([D, BC], FP32, tag="u1")
        nc.vector.tensor_tensor(out=u1, in0=o01[:, :, 0], in1=r0bc,
                                op=ALU.mult)
        nc.vector.tensor_tensor(out=u1, in0=u1, in1=ub0, op=ALU.add)

        ub1 = lin_relu(u1, 1)
        c2 = make_c(ub1, o1wu, 2)
        t2bc, corr2 = hop_scalars(c2, 2)
        num2 = sb.tile([128, CH, BC], BF16, tag="num2")
        make_num(t2bc, num2[:, :, :], 2)
        d2, recip2 = denom(num2, corr2, 2)

        r1bc = bcast_row(recip1, "r1")
        u2 = sb.tile([D, BC], FP32, tag="u2")
        nc.vector.tensor_tensor(out=u2, in0=o01[:, :, 1], in1=r1bc,
                                op=ALU.mult)
        nc.vector.tensor_tensor(out=u2, in0=u2, in1=ub1, op=ALU.add)
        ub2 = lin_relu(u2, 2)

        # ---- pass B: o2 ----
        o2 = sb.tile([128, BC], FP32, tag="o2")
        for b in range(BC):
            acc = ps_acc.tile([1, 128], FP32, tag="acc")
            for c in range(CH):
                nc.tensor.matmul(
                    acc, lhsT=num2[:, c, b:b + 1], rhs=v_sb[b][:, c, :],
                    start=(c == 0), stop=(c == CH - 1))
            ob_sb = wk.tile([1, 128], FP32, tag="ob")
            if b % 2 == 0:
                nc.vector.tensor_copy(ob_sb, acc)
            else:
                nc.scalar.activation(out=ob_sb, in_=acc, func=AF.Copy)
            ps_t = ps_sm.tile([128, 1], FP32, tag="smb")
            nc.tensor.transpose(out=ps_t, in_=ob_sb,
                                identity=identf[0:1, 0:1])
            nc.vector.tensor_copy(o2[:, b:b + 1], ps_t)

        # ---- u3 and output ----
        r2bc = bcast_row(recip2, "r2")
        u3 = sb.tile([D, BC], FP32, tag="u3")
        nc.vector.tensor_tensor(out=u3, in0=o2, in1=r2bc, op=ALU.mult)
        nc.vector.tensor_tensor(out=u3, in0=u3, in1=ub2, op=ALU.add)

        ps_y = ps_sm.tile([BC, 128], FP32, tag="smb")
        nc.tensor.transpose(out=ps_y, in_=u3, identity=identf)
        y_sb = wk.tile([BC, 128], FP32, tag="ysb")
        nc.vector.tensor_copy(y_sb, ps_y)
        nc.sync.dma_start(out=y[:, :], in_=y_sb)

    _split_multiwaits(nc)
    return nc


_nc_cache = None


def _get_nc():
    global _nc_cache
    if _nc_cache is None:
        _nc_cache = _build()
    return _nc_cache


def kernel(**inputs):
    e1 = np.asarray(inputs["e1_embeded"], dtype=np.float32)
    value = np.asarray(inputs["nei_embeded_value"], dtype=np.float32)
    mask = np.asarray(inputs["nei_mask"], dtype=np.float32)
    linfc_w = np.asarray(inputs["linfc_w"], dtype=np.float32)
    linfc_b = np.asarray(inputs["linfc_b"], dtype=np.float32)
    attfc_w = np.asarray(inputs["attfc_w"], dtype=np.float32)
    attfc_b = np.asarray(inputs["attfc_b"], dtype=np.float32)

    w_lhsT = np.ascontiguousarray(linfc_w.T)
    b_col = np.ascontiguousarray(linfc_b.reshape(D, 1))
    wfu = np.ascontiguousarray(
        np.stack([attfc_w[0, :D], attfc_w[0, D:]], axis=1))
    attb = np.asarray(attfc_b, dtype=np.float32).reshape(1, 1)
    ident = np.eye(128, dtype=np.float32)

    in_maps = []
    for core in range(N_CORES):
        b0 = core * BC
        in_maps.append({
            "value": np.ascontiguousarray(value[b0:b0 + BC]),
            "mask_t": np.ascontiguousarray(mask[b0:b0 + BC].T),
            "e1_t": np.ascontiguousarray(e1[b0:b0 + BC].T),
            "w_lhsT": w_lhsT,
            "b_col": b_col,
            "wfu": wfu,
            "attb": attb,
            "ident": ident,
        })

    nc = _get_nc()
    res = run_bass_kernel_spmd(nc, in_maps, list(range(N_CORES)))
    out = np.concatenate([res.results[i]["y"] for i in range(N_CORES)], axis=0)
    return out.astype(np.float32)
